# revision 5
# baseline (speedup 1.0000x reference)
"""AdaptiveGraphAttention Trainium2 kernel (8 NeuronCores, data-parallel).

Math: in the reference, logits[b,h,i,j] = a_q[b,h,i] + a_k[b,h,j] +
e_j[b,h,j]*adj[i,j] + attn_b with adj[:,0]=0, adj[:,1:]=1 — the mask and the
j-dependent terms are identical for every query row i, and the a_q/bias terms
are constant over j.  Softmax is shift-invariant, so the attention
distribution p[b,h,:] = softmax_{j>=1}(a_k + e_j) is the same for all i: the
attention matrix is rank-1 and the output is one row per batch, broadcast
over the 256 query positions.  bq/bk/attn_b cancel exactly; bv survives as
an additive constant (sum_j p_j = 1); bv and bo are folded on the host.

Per-head dots fold into small matrices:
  a_k[b,j,h] = nv[b,j,:] @ Uk[:,h],  Uk[d,h] = sum_m Wk[h*64+m, d] * w_k[m]
  e_j[b,j,h] = desc[b,j-1,:] @ Ue[:,h], Ue[h*64+m, h] = w_e(m) (else 0)

Device work per core (4 batches), all cross-core-communication-free:
  psc_b[h,j]  = Uk.T @ nvT[:,j] + Ue.T @ descT[:,j-1]   (PE DoubleRow fp8)
  p_b[h,:]    = softmax_j(psc_b)            (ACT exp+accum, DVE recip/mul)
  pT[j,(b,h)] = XBAR DMA transpose of the four p_b tiles
  nvbar       = pT_b.T-stationary @ nv_b    -> PSUM [128=(4b pad 32), 512]x2
  nvh/nvbT    = cast + XBAR DMA transpose   -> [128 d, 8 chunk, (b pad) h]
  VbarT       = WvT @ nvbT  (full d', selected to ctxT by blockdiag copy)
  out         = ctxT.T @ WoT -> [4, 1024]                         -> DMA
bv is folded into the host-side output bias (out += Wo @ bv + bo, exact
since sum_j p_j = 1).

All DRAM inputs are host-prepermuted to [128, chunk, inner] so each DMA
partition row is one contiguous run.
"""

import numpy as np
import ml_dtypes
from contextlib import ExitStack

import concourse.bass as bass
import concourse.mybir as mybir
import concourse.tile as tile
from concourse import bacc
from concourse.bass_utils import run_bass_kernel_spmd

B, S, D, H, HD = 32, 256, 1024, 16, 64
NCORES = 8
BPC = B // NCORES  # 4 batches per core
F32 = mybir.dt.float32
BF16 = mybir.dt.bfloat16
NPBF = ml_dtypes.bfloat16
F8 = mybir.dt.float8e4
NPF8 = ml_dtypes.float8_e4m3
USCALE = 512.0  # fp8 range lift for the tiny folded U entries
DC = D // 128  # 8 chunks of the model dim
JC = S // 128  # 2 chunks of the sequence dim

_cache = {}


def _build():
    nc = bacc.Bacc("TRN2", target_bir_lowering=False, debug=False,
                   num_devices=NCORES)

    nv_ext = nc.declare_dram_parameter("nv", [BPC, 128, JC, D], BF16,
                                       isOutput=False)
    xt_ext = nc.declare_dram_parameter("xT", [BPC, 128, DC, 2 * S], F8,
                                       isOutput=False)
    u_ext = nc.declare_dram_parameter("U", [128, DC, 2 * H], F8,
                                      isOutput=False)
    wvt_ext = nc.declare_dram_parameter("WvT", [128, DC, DC, 128], BF16,
                                        isOutput=False)
    wot_ext = nc.declare_dram_parameter("WoT", [128, DC, D], BF16,
                                        isOutput=False)
    out_ext = nc.declare_dram_parameter("out", [BPC, D], F32, isOutput=True)

    DR = mybir.MatmulPerfMode.DoubleRow
    EXPF = mybir.ActivationFunctionType.Exp

    with tile.TileContext(nc) as tc, ExitStack() as ctx:
        wpool = ctx.enter_context(tc.tile_pool(name="w", bufs=1))
        xpool = ctx.enter_context(tc.tile_pool(name="x", bufs=4))
        smpool = ctx.enter_context(tc.tile_pool(name="sm", bufs=2))
        ps_c = ctx.enter_context(tc.tile_pool(name="ps_c", bufs=2, space="PSUM"))
        ps_nb = ctx.enter_context(tc.tile_pool(name="ps_nb", bufs=1, space="PSUM"))
        ps_vb = ctx.enter_context(tc.tile_pool(name="ps_vb", bufs=2, space="PSUM"))
        ps_o = ctx.enter_context(tc.tile_pool(name="ps_o", bufs=1, space="PSUM"))

        u_sb = wpool.tile([128, DC, 2 * H], F8)
        nc.sync.dma_start(out=u_sb[:], in_=u_ext.ap())

        # activation DMAs, batch-interleaved so per-batch compute can start
        xt_sb, nv_sb = [], []
        for b in range(BPC):
            xt = xpool.tile([128, DC, 2 * S], F8)
            nc.sync.dma_start(out=xt[:], in_=xt_ext[b])
            nv = xpool.tile([128, JC, D], BF16)
            nc.sync.dma_start(out=nv[:], in_=nv_ext[b])
            xt_sb.append(xt)
            nv_sb.append(nv)

        # weights after activations; chunked so phase-2 matmuls pipeline
        # behind the arriving chunks
        wvt_sb = wpool.tile([128, DC, DC, 128], BF16)
        for cm in range(DC):
            nc.sync.dma_start(out=wvt_sb[:, cm], in_=wvt_ext[:, cm])
        wot_sb = wpool.tile([128, DC, D], BF16)
        for ck in range(DC):
            nc.sync.dma_start(out=wot_sb[:, ck], in_=wot_ext[:, ck])

        # --- phase 1: logits -> softmax -> pT -> nvbar ---------------------
        pt_sb = smpool.tile([128, JC, BPC * H], BF16)
        nbA = ps_nb.tile([128, 512], F32)
        nbB = ps_nb.tile([128, 512], F32)
        for b in range(BPC):
            psc = ps_c.tile([H, S], F32)
            for c2 in range(DC // 2):
                pair = slice(2 * c2, 2 * c2 + 2)
                nc.tensor.matmul(psc[:, 1:S], u_sb[:, pair, 0:H],
                                 xt_sb[b][:, pair, 1:S],
                                 start=(c2 == 0), stop=False, perf_mode=DR)
            for c2 in range(DC // 2):
                pair = slice(2 * c2, 2 * c2 + 2)
                nc.tensor.matmul(psc[:, 1:S], u_sb[:, pair, H:2 * H],
                                 xt_sb[b][:, pair, S:2 * S - 1],
                                 start=False, stop=(c2 == DC // 2 - 1),
                                 perf_mode=DR)

            # softmax over j (free dim); logits are O(1), no max-subtraction
            p2 = smpool.tile([H, S], BF16)
            nc.gpsimd.memset(p2[:, 0:1], 0.0)
            sumx = smpool.tile([H, 1], F32)
            nc.scalar.activation(p2[:, 1:S], psc[:, 1:S], EXPF,
                                 scale=1.0 / USCALE, accum_out=sumx[:])
            recip = smpool.tile([H, 1], F32)
            nc.vector.reciprocal(recip[:], sumx[:])
            nc.vector.tensor_scalar_mul(p2[:, 1:S], p2[:, 1:S], recip[:])

            # pT[j, (b,h)] via XBAR DMA transpose (no PE, no PSUM)
            nc.sync.dma_start_transpose(
                pt_sb[:, :, b * H:(b + 1) * H], p2[:])

            # nvbar[(32b+h), d] = sum_j p[j,(b,h)] nv[j, d]; batches stacked
            # at 32-row offsets (matmul tile_position wants multiples of 32)
            for jc in range(JC):
                st = pt_sb[:, jc, b * H:(b + 1) * H]
                nc.tensor.matmul(nbA[32 * b:32 * b + H, :], st,
                                 nv_sb[b][:, jc, 0:512],
                                 start=(jc == 0), stop=(jc == JC - 1),
                                 tile_position=(0, 32 * b))
                nc.tensor.matmul(nbB[32 * b:32 * b + H, :], st,
                                 nv_sb[b][:, jc, 512:1024],
                                 start=(jc == 0), stop=(jc == JC - 1),
                                 tile_position=(0, 32 * b))

        nvh = smpool.tile([128, D], BF16)
        nc.vector.tensor_copy(nvh[:, 0:512], nbA[:])
        nc.vector.tensor_copy(nvh[:, 512:1024], nbB[:])
        # nvbT[d%128, dchunk, (32b+h)]
        nvbT = smpool.tile([128, DC, 128], BF16)
        nc.sync.dma_start_transpose(nvbT[:], nvh[:])

        # --- phase 2: VbarT (full d') -> blockdiag select -> out -----------
        ctx_sb = wpool.tile([128, DC, BPC], BF16)
        for cm in range(DC):
            vb = ps_vb.tile([128, BPC * H], F32)
            for ck in range(DC):
                mov = nvbT[:, ck, :].rearrange(
                    "p (b h) -> p b h", h=2 * H)[:, :, 0:H]
                nc.tensor.matmul(vb[:], wvt_sb[:, cm, ck, :], mov,
                                 start=(ck == 0), stop=(ck == DC - 1))
            for half in range(2):
                h = 2 * cm + half
                rows = slice(64 * half, 64 * half + 64)
                s_ap = vb[rows, :].rearrange("p (b h) -> p b h", h=H)[:, :, h]
                nc.vector.tensor_copy(ctx_sb[rows, cm, :], s_ap)

        oA = ps_o.tile([BPC, 512], F32)
        oB = ps_o.tile([BPC, 512], F32)
        o_ps = [oA, oB]
        for ck in range(DC):
            for n2 in range(2):
                cols = slice(n2 * 512, (n2 + 1) * 512)
                nc.tensor.matmul(o_ps[n2][:], ctx_sb[:, ck, :],
                                 wot_sb[:, ck, cols],
                                 start=(ck == 0), stop=(ck == DC - 1))
        o_sb = smpool.tile([BPC, D], F32)
        nc.vector.tensor_copy(o_sb[:, 0:512], o_ps[0][:])
        nc.scalar.activation(o_sb[:, 512:1024], o_ps[1][:],
                             mybir.ActivationFunctionType.Copy)
        nc.sync.dma_start(out=out_ext.ap(), in_=o_sb[:])

    nc.compile()
    return nc


def _prep(desc, nv, Wk, Wv, Wo, attn_w):
    w_k = attn_w[HD:2 * HD]
    w_e = attn_w[2 * HD:]
    Uk = np.einsum('hmd,m->dh', Wk.reshape(H, HD, D), w_k)
    Ue = np.zeros((D, H), np.float32)
    for h in range(H):
        Ue[h * HD:(h + 1) * HD, h] = w_e
    U = np.concatenate([Uk, Ue], axis=1) * USCALE           # [D, 32]
    Up = np.ascontiguousarray(
        U.reshape(DC, 128, 2 * H).swapaxes(0, 1)).astype(NPF8)
    WvTp = np.ascontiguousarray(
        Wv.T.reshape(DC, 128, DC, 128).transpose(1, 2, 0, 3)).astype(NPBF)
    WoTp = np.ascontiguousarray(
        Wo.T.reshape(DC, 128, D).swapaxes(0, 1)).astype(NPBF)
    # nv natural, chunked over j: [B, 128, JC, D]
    nvp = np.ascontiguousarray(
        nv.reshape(B, JC, 128, D).swapaxes(1, 2)).astype(NPBF)
    # nv transposed, chunked over d: [B, 128, DC, S]
    nvTp = nv.transpose(0, 2, 1).reshape(B, DC, 128, S).swapaxes(1, 2)
    descTp = desc.transpose(0, 2, 1).reshape(B, DC, 128, S - 1).swapaxes(1, 2)
    pad = np.zeros((B, 128, DC, 1), np.float32)
    xTp = np.concatenate([nvTp, descTp, pad], axis=3).astype(NPF8)
    return Up, WvTp, WoTp, nvp, xTp


def kernel(desc_embeddings, name_value_embeddings, Wq, bq, Wk, bk, Wv, bv,
           attn_w, attn_b, Wo, bo, _trace=False):
    desc = np.asarray(desc_embeddings, np.float32)
    nv = np.asarray(name_value_embeddings, np.float32)
    Up, WvTp, WoTp, nvp, xTp = _prep(
        desc, nv, np.asarray(Wk, np.float32), np.asarray(Wv, np.float32),
        np.asarray(Wo, np.float32), np.asarray(attn_w, np.float32))

    if "nc" not in _cache:
        _cache["nc"] = _build()
    nc = _cache["nc"]

    in_maps = []
    for c in range(NCORES):
        sl = slice(c * BPC, (c + 1) * BPC)
        in_maps.append({
            "nv": np.ascontiguousarray(nvp[sl]),
            "xT": np.ascontiguousarray(xTp[sl]),
            "U": Up, "WvT": WvTp, "WoT": WoTp,
        })
    res = run_bass_kernel_spmd(nc, in_maps, core_ids=list(range(NCORES)),
                               trace=_trace)
    out_rows = np.empty((B, D), np.float32)
    for c in range(NCORES):
        out_rows[c * BPC:(c + 1) * BPC] = res.results[c]["out"]
    bo_eff = (np.asarray(bo, np.float32)
              + np.asarray(Wo, np.float32) @ np.asarray(bv, np.float32))
    out_rows += bo_eff[None, :]
    full = np.broadcast_to(out_rows[:, None, :], (B, S, D))
    if _trace:
        return np.ascontiguousarray(full), res
    return np.ascontiguousarray(full)
